# revision 3
# baseline (speedup 1.0000x reference)
"""Trainium2 Bass kernel for nn_KKLayer (spectral channel-mix layer).

Math identity: the reference computes
    y = Re(IFFT2((A + iB) . conj(FFT2(x))))            (channel mix in freq domain)
Since channel mixing commutes with the spatial FFT and, for real x,
IFFT2(conj(FFT2(x))) is x spatially "negated" (h -> (-h) mod H, w -> (-w) mod W),
the whole layer collapses to
    y[b,o,h,w] = sum_i A[o,i] * x[b,i,(H-h)%H,(W-w)%W]
(betas drop out of the real part entirely).

The (h,w) flip is folded into the host-side shard step (a fancy-index while
casting x to bf16), so the device kernel is a pure streaming channel-matmul:

  per core (data-parallel over batch, 8 batches -> 8 cores):
    - load alphas^T (stationary weights, bf16) + pre-flipped x[b] (bf16)
    - per 2048-col chunk: 4 bf16 matmuls [K=128,M=128,N=512] -> PSUM fp32,
      one [128,2048] PSUM->SBUF copy (fp32 -> bf16, alternating DVE/ACT),
      one contiguous 512KB bf16 DMA out
    - host upcasts bf16 -> fp32

bf16 I/O halves HBM traffic (8.4MB/core, ~23.5us at the 358GB/s/core limit)
and runs the PE at 1 cycle/row instead of fp32's 4 (rel err ~4e-3 << 2e-2).
"""

import ml_dtypes
import numpy as np

import concourse.bass as bass
import concourse.bacc as bacc
import concourse.mybir as mybir
from concourse import tile
from concourse.bass_utils import run_bass_kernel_spmd

B, CIN, COUT, H, W = 8, 128, 128, 128, 128
HW = H * W          # 16384
BLK = 512           # matmul free dim (one PSUM bank of fp32)
CHUNK = 2048        # cols per pipeline chunk (4 matmul blocks, 512KB bf16)
NCHUNK = HW // CHUNK
N_CORES = 8

F32 = mybir.dt.float32
BF16 = mybir.dt.bfloat16
NP_BF16 = ml_dtypes.bfloat16

# (-h) % H index for the host-side spatial flip
_FLIP = (-np.arange(H)) % H


def _build_nc():
    nc = bacc.Bacc(None, target_bir_lowering=False)
    x = nc.dram_tensor("x", [CIN, HW], BF16, kind="ExternalInput")
    wT = nc.dram_tensor("wT", [CIN, COUT], BF16, kind="ExternalInput")
    y = nc.dram_tensor("y", [COUT, HW], BF16, kind="ExternalOutput")

    with tile.TileContext(nc) as tc:
        with (
            tc.tile_pool(name="wp", bufs=1) as wpool,
            tc.tile_pool(name="xp", bufs=1) as xpool,
            tc.tile_pool(name="yp", bufs=3) as ypool,
            tc.tile_pool(name="ps", bufs=2, space="PSUM") as pspool,
        ):
            w_t = wpool.tile([CIN, COUT], BF16)
            nc.sync.dma_start(w_t[:], wT[:])

            # all input chunks up front: the HWDGE ring drains them FIFO,
            # so chunk k lands ~k*1.4us in and compute pipelines behind
            xch = []
            for k in range(NCHUNK):
                t = xpool.tile([CIN, CHUNK], BF16, tag=f"x{k}", name=f"xch{k}")
                nc.sync.dma_start(t[:], x[:, CHUNK * k: CHUNK * (k + 1)])
                xch.append(t)

            for k in range(NCHUNK):
                ps = pspool.tile([COUT, CHUNK], F32, tag="ps", name=f"ps{k}")
                for j in range(CHUNK // BLK):
                    nc.tensor.matmul(
                        ps[:, BLK * j: BLK * (j + 1)],
                        w_t[:],
                        xch[k][:, BLK * j: BLK * (j + 1)],
                        start=True,
                        stop=True,
                    )
                yt = ypool.tile([COUT, CHUNK], BF16, tag="y", name=f"ych{k}")
                # one big PSUM->SBUF downconvert copy; alternate engines so
                # consecutive chunks' copies overlap
                if k % 2 == 0:
                    nc.vector.tensor_copy(yt[:], ps[:])
                else:
                    nc.scalar.copy(yt[:], ps[:])
                nc.sync.dma_start(y[:, CHUNK * k: CHUNK * (k + 1)], yt[:])
    nc.compile()
    return nc


_NC_CACHE = {}


def _get_nc():
    if "nc" not in _NC_CACHE:
        _NC_CACHE["nc"] = _build_nc()
    return _NC_CACHE["nc"]


def make_in_maps(x, alphas):
    """Per-core input maps: bf16, with the (h,w) flip pre-applied to x."""
    x16 = np.asarray(x, dtype=np.float32).astype(NP_BF16)
    wT = np.ascontiguousarray(
        np.asarray(alphas, dtype=np.float32).T
    ).astype(NP_BF16)
    maps = []
    for c in range(N_CORES):
        xf = x16[c][:, _FLIP][:, :, _FLIP]
        maps.append(
            {"x": np.ascontiguousarray(xf.reshape(CIN, HW)), "wT": wT}
        )
    return maps


def kernel(x, alphas, betas=None, **_unused):
    nc = _get_nc()
    in_maps = make_in_maps(x, alphas)
    res = run_bass_kernel_spmd(nc, in_maps, core_ids=list(range(N_CORES)))
    out = np.stack(
        [
            res.results[c]["y"].astype(np.float32).reshape(COUT, H, W)
            for c in range(N_CORES)
        ]
    )
    return out


# revision 5
# speedup vs baseline: 1.1331x; 1.1331x over previous
"""Trainium2 Bass kernel for nn_KKLayer (spectral channel-mix layer).

Math identity: the reference computes
    y = Re(IFFT2((A + iB) . conj(FFT2(x))))            (channel mix in freq domain)
Since channel mixing commutes with the spatial FFT and, for real x,
IFFT2(conj(FFT2(x))) is x spatially "negated" (h -> (-h) mod H, w -> (-w) mod W),
the whole layer collapses to
    y[b,o,h,w] = sum_i A[o,i] * x[b,i,(H-h)%H,(W-w)%W]
(betas drop out of the real part entirely).

The (h,w) flip is folded into the host-side shard step (a fancy-index while
casting x to bf16), so the device kernel is a pure streaming channel-matmul:

  per core (data-parallel over batch, 8 batches -> 8 cores):
    - load alphas^T (stationary weights, bf16) + pre-flipped x[b] (bf16)
    - per 2048-col chunk: 4 bf16 matmuls [K=128,M=128,N=512] -> PSUM fp32,
      one [128,2048] PSUM->SBUF copy (fp32 -> bf16, alternating DVE/ACT),
      one contiguous 512KB bf16 DMA out
    - host upcasts bf16 -> fp32

bf16 I/O halves HBM traffic (8.4MB/core, ~23.5us at the 358GB/s/core limit)
and runs the PE at 1 cycle/row instead of fp32's 4 (rel err ~4e-3 << 2e-2).
"""

import ml_dtypes
import numpy as np

import concourse.bass as bass
import concourse.bacc as bacc
import concourse.mybir as mybir
from concourse import tile
from concourse.bass_utils import run_bass_kernel_spmd

B, CIN, COUT, H, W = 8, 128, 128, 128, 128
HW = H * W          # 16384
BLK = 512           # matmul free dim (one PSUM bank of fp32)
# col counts per pipeline chunk: small first chunk so compute starts early
CHUNK_COLS = [512, 1536] + [2048] * 7
N_CORES = 8

F32 = mybir.dt.float32
BF16 = mybir.dt.bfloat16
NP_BF16 = ml_dtypes.bfloat16

# (-h) % H index for the host-side spatial flip
_FLIP = (-np.arange(H)) % H


def _build_nc():
    nc = bacc.Bacc(None, target_bir_lowering=False)
    x = nc.dram_tensor("x", [CIN, HW], BF16, kind="ExternalInput")
    wT = nc.dram_tensor("wT", [CIN, COUT], BF16, kind="ExternalInput")
    y = nc.dram_tensor("y", [COUT, HW], BF16, kind="ExternalOutput")

    offs = np.cumsum([0] + CHUNK_COLS)
    with tile.TileContext(nc) as tc:
        with (
            tc.tile_pool(name="wp", bufs=1) as wpool,
            tc.tile_pool(name="xp", bufs=1) as xpool,
            tc.tile_pool(name="yp", bufs=1) as ypool,
            tc.tile_pool(name="ps", bufs=2, space="PSUM") as pspool,
        ):
            # all input chunks up front: the HWDGE ring drains them FIFO, so
            # chunk k lands ~k*1.4us in and compute pipelines behind.  x0
            # goes before w so the first matmul's input is in flight first.
            xch = []
            for k, cols in enumerate(CHUNK_COLS):
                t = xpool.tile([CIN, cols], BF16, tag=f"x{k}", name=f"xch{k}")
                nc.sync.dma_start(t[:], x[:, offs[k]: offs[k + 1]])
                xch.append(t)
                if k == 0:
                    w_t = wpool.tile([CIN, COUT], BF16)
                    nc.sync.dma_start(w_t[:], wT[:])

            for k, cols in enumerate(CHUNK_COLS):
                ps = pspool.tile([COUT, 2048], F32, tag="ps", name=f"ps{k}")
                for j in range(cols // BLK):
                    nc.tensor.matmul(
                        ps[:, BLK * j: BLK * (j + 1)],
                        w_t[:],
                        xch[k][:, BLK * j: BLK * (j + 1)],
                        start=True,
                        stop=True,
                    )
                # dedicated y tile per chunk: the copy never waits on a
                # previous out-DMA (no write-after-read coupling)
                yt = ypool.tile([COUT, cols], BF16, tag=f"y{k}", name=f"ych{k}")
                # one big PSUM->SBUF downconvert copy; alternate engines so
                # consecutive chunks' copies overlap
                if k % 2 == 0:
                    nc.vector.tensor_copy(yt[:], ps[:, 0:cols])
                else:
                    nc.scalar.copy(yt[:], ps[:, 0:cols])
                nc.sync.dma_start(y[:, offs[k]: offs[k + 1]], yt[:])
    nc.compile()
    return nc


_NC_CACHE = {}


def _get_nc():
    if "nc" not in _NC_CACHE:
        _NC_CACHE["nc"] = _build_nc()
    return _NC_CACHE["nc"]


def make_in_maps(x, alphas):
    """Per-core input maps: bf16, with the (h,w) flip pre-applied to x."""
    x16 = np.asarray(x, dtype=np.float32).astype(NP_BF16)
    wT = np.ascontiguousarray(
        np.asarray(alphas, dtype=np.float32).T
    ).astype(NP_BF16)
    maps = []
    for c in range(N_CORES):
        xf = x16[c][:, _FLIP][:, :, _FLIP]
        maps.append(
            {"x": np.ascontiguousarray(xf.reshape(CIN, HW)), "wT": wT}
        )
    return maps


def kernel(x, alphas, betas=None, **_unused):
    nc = _get_nc()
    in_maps = make_in_maps(x, alphas)
    res = run_bass_kernel_spmd(nc, in_maps, core_ids=list(range(N_CORES)))
    out = np.stack(
        [
            res.results[c]["y"].astype(np.float32).reshape(COUT, H, W)
            for c in range(N_CORES)
        ]
    )
    return out


# revision 7
# speedup vs baseline: 1.1432x; 1.0089x over previous
"""Trainium2 Bass kernel for nn_KKLayer (spectral channel-mix layer).

Math identity: the reference computes
    y = Re(IFFT2((A + iB) . conj(FFT2(x))))            (channel mix in freq domain)
Since channel mixing commutes with the spatial FFT and, for real x,
IFFT2(conj(FFT2(x))) is x spatially "negated" (h -> (-h) mod H, w -> (-w) mod W),
the whole layer collapses to
    y[b,o,h,w] = sum_i A[o,i] * x[b,i,(H-h)%H,(W-w)%W]
(betas drop out of the real part entirely).

The (h,w) flip is folded into the host-side shard step (a fancy-index while
casting x to bf16), so the device kernel is a pure streaming channel-matmul:

  per core (data-parallel over batch, 8 batches -> 8 cores):
    - load alphas^T (stationary weights, bf16) + pre-flipped x[b] (bf16)
    - per 2048-col chunk: 4 bf16 matmuls [K=128,M=128,N=512] -> PSUM fp32,
      one [128,2048] PSUM->SBUF copy (fp32 -> bf16, alternating DVE/ACT),
      one contiguous 512KB bf16 DMA out
    - host upcasts bf16 -> fp32

bf16 I/O halves HBM traffic (8.4MB/core, ~23.5us at the 358GB/s/core limit)
and runs the PE at 1 cycle/row instead of fp32's 4 (rel err ~4e-3 << 2e-2).
"""

import ml_dtypes
import numpy as np

import concourse.bass as bass
import concourse.bacc as bacc
import concourse.mybir as mybir
from concourse import tile
from concourse.bass_utils import run_bass_kernel_spmd

B, CIN, COUT, H, W = 8, 128, 128, 128, 128
HW = H * W          # 16384
BLK = 512           # matmul free dim (one PSUM bank of fp32)
# col counts per pipeline chunk: small first chunk so compute starts early,
# tapered tail so the last out-DMAs aren't gated on long compute chains
CHUNK_COLS = [512, 1536] + [2048] * 5 + [1536, 1024, 768, 512, 256]
N_CORES = 8

F32 = mybir.dt.float32
BF16 = mybir.dt.bfloat16
NP_BF16 = ml_dtypes.bfloat16

# (-h) % H index for the host-side spatial flip
_FLIP = (-np.arange(H)) % H


def _build_nc():
    nc = bacc.Bacc(None, target_bir_lowering=False)
    x = nc.dram_tensor("x", [CIN, HW], BF16, kind="ExternalInput")
    wT = nc.dram_tensor("wT", [CIN, COUT], BF16, kind="ExternalInput")
    y = nc.dram_tensor("y", [COUT, HW], BF16, kind="ExternalOutput")

    offs = np.cumsum([0] + CHUNK_COLS)
    with tile.TileContext(nc) as tc:
        with (
            tc.tile_pool(name="wp", bufs=1) as wpool,
            tc.tile_pool(name="xp", bufs=1) as xpool,
            tc.tile_pool(name="yp", bufs=1) as ypool,
            tc.tile_pool(name="ps", bufs=2, space="PSUM") as pspool,
        ):
            # all input chunks up front: the HWDGE ring drains them FIFO, so
            # chunk k lands ~k*1.4us in and compute pipelines behind.  x0
            # goes before w so the first matmul's input is in flight first.
            xch = []
            for k, cols in enumerate(CHUNK_COLS):
                t = xpool.tile([CIN, cols], BF16, tag=f"x{k}", name=f"xch{k}")
                nc.sync.dma_start(t[:], x[:, offs[k]: offs[k + 1]])
                xch.append(t)
                if k == 0:
                    w_t = wpool.tile([CIN, COUT], BF16)
                    nc.sync.dma_start(w_t[:], wT[:])

            for k, cols in enumerate(CHUNK_COLS):
                ps = pspool.tile([COUT, 2048], F32, tag="ps", name=f"ps{k}")
                for j0 in range(0, cols, BLK):
                    j1 = min(j0 + BLK, cols)
                    nc.tensor.matmul(
                        ps[:, j0:j1],
                        w_t[:],
                        xch[k][:, j0:j1],
                        start=True,
                        stop=True,
                    )
                # dedicated y tile per chunk: the copy never waits on a
                # previous out-DMA (no write-after-read coupling)
                yt = ypool.tile([COUT, cols], BF16, tag=f"y{k}", name=f"ych{k}")
                # one big PSUM->SBUF downconvert copy; alternate engines so
                # consecutive chunks' copies overlap
                if k % 2 == 0:
                    nc.vector.tensor_copy(yt[:], ps[:, 0:cols])
                else:
                    nc.scalar.copy(yt[:], ps[:, 0:cols])
                nc.sync.dma_start(y[:, offs[k]: offs[k + 1]], yt[:])
    nc.compile()
    return nc


_NC_CACHE = {}


def _get_nc():
    if "nc" not in _NC_CACHE:
        _NC_CACHE["nc"] = _build_nc()
    return _NC_CACHE["nc"]


def make_in_maps(x, alphas):
    """Per-core input maps: bf16, with the (h,w) flip pre-applied to x."""
    x16 = np.asarray(x, dtype=np.float32).astype(NP_BF16)
    wT = np.ascontiguousarray(
        np.asarray(alphas, dtype=np.float32).T
    ).astype(NP_BF16)
    maps = []
    for c in range(N_CORES):
        xf = x16[c][:, _FLIP][:, :, _FLIP]
        maps.append(
            {"x": np.ascontiguousarray(xf.reshape(CIN, HW)), "wT": wT}
        )
    return maps


def kernel(x, alphas, betas=None, **_unused):
    nc = _get_nc()
    in_maps = make_in_maps(x, alphas)
    res = run_bass_kernel_spmd(nc, in_maps, core_ids=list(range(N_CORES)))
    out = np.stack(
        [
            res.results[c]["y"].astype(np.float32).reshape(COUT, H, W)
            for c in range(N_CORES)
        ]
    )
    return out


# revision 9
# speedup vs baseline: 1.1539x; 1.0094x over previous
"""Trainium2 Bass kernel for nn_KKLayer (spectral channel-mix layer).

Math identity: the reference computes
    y = Re(IFFT2((A + iB) . conj(FFT2(x))))            (channel mix in freq domain)
Since channel mixing commutes with the spatial FFT and, for real x,
IFFT2(conj(FFT2(x))) is x spatially "negated" (h -> (-h) mod H, w -> (-w) mod W),
the whole layer collapses to
    y[b,o,h,w] = sum_i A[o,i] * x[b,i,(H-h)%H,(W-w)%W]
(betas drop out of the real part entirely).

The (h,w) flip is folded into the host-side shard step (a fancy-index while
casting x to bf16), so the device kernel is a pure streaming channel-matmul:

  per core (data-parallel over batch, 8 batches -> 8 cores):
    - load alphas^T (stationary weights, bf16) + pre-flipped x[b] (bf16)
    - per 2048-col chunk: 4 bf16 matmuls [K=128,M=128,N=512] -> PSUM fp32,
      one [128,2048] PSUM->SBUF copy (fp32 -> bf16, alternating DVE/ACT),
      one contiguous 512KB bf16 DMA out
    - host upcasts bf16 -> fp32

bf16 I/O halves HBM traffic (8.4MB/core, ~23.5us at the 358GB/s/core limit)
and runs the PE at 1 cycle/row instead of fp32's 4 (rel err ~4e-3 << 2e-2).
"""

import ml_dtypes
import numpy as np

import concourse.bass as bass
import concourse.bacc as bacc
import concourse.mybir as mybir
from concourse import tile
from concourse.bass_utils import run_bass_kernel_spmd

B, CIN, COUT, H, W = 8, 128, 128, 128, 128
HW = H * W          # 16384
BLK = 512           # matmul free dim (one PSUM bank of fp32)
# input DMA sizes: few big transfers keep the SDMA engines at line rate
# (~425GB/s); small transfers are latency-dominated and waste bus time
IN_COLS = [2048, 4096, 4096, 4096, 2048]
# compute/copy/out pipeline granularity (cols): one PSUM allocation (4 banks)
CHUNK = 2048
N_CORES = 8

F32 = mybir.dt.float32
BF16 = mybir.dt.bfloat16
NP_BF16 = ml_dtypes.bfloat16

# (-h) % H index for the host-side spatial flip
_FLIP = (-np.arange(H)) % H


def _build_nc():
    nc = bacc.Bacc(None, target_bir_lowering=False)
    x = nc.dram_tensor("x", [CIN, HW], BF16, kind="ExternalInput")
    wT = nc.dram_tensor("wT", [CIN, COUT], BF16, kind="ExternalInput")
    y = nc.dram_tensor("y", [COUT, HW], BF16, kind="ExternalOutput")

    in_offs = np.cumsum([0] + IN_COLS)
    with tile.TileContext(nc) as tc:
        with (
            tc.tile_pool(name="wp", bufs=1) as wpool,
            tc.tile_pool(name="xp", bufs=1) as xpool,
            tc.tile_pool(name="yp", bufs=1) as ypool,
            tc.tile_pool(name="ps", bufs=2, space="PSUM") as pspool,
        ):
            # all input DMAs up front: the HWDGE ring drains them FIFO at
            # line rate; compute is bus-hidden so it just follows along.
            # x0 goes before w so the first matmul's input is in flight first.
            xin = []
            for k, cols in enumerate(IN_COLS):
                t = xpool.tile([CIN, cols], BF16, tag=f"x{k}", name=f"xch{k}")
                nc.sync.dma_start(t[:], x[:, in_offs[k]: in_offs[k + 1]])
                xin.append(t)
                if k == 0:
                    w_t = wpool.tile([CIN, COUT], BF16)
                    nc.sync.dma_start(w_t[:], wT[:])

            for c in range(HW // CHUNK):
                base = c * CHUNK
                # which input tile holds this chunk's columns
                k = int(np.searchsorted(in_offs, base, side="right")) - 1
                lo = base - in_offs[k]
                ps = pspool.tile([COUT, CHUNK], F32, tag="ps", name=f"ps{c}")
                for j in range(CHUNK // BLK):
                    nc.tensor.matmul(
                        ps[:, BLK * j: BLK * (j + 1)],
                        w_t[:],
                        xin[k][:, lo + BLK * j: lo + BLK * (j + 1)],
                        start=True,
                        stop=True,
                    )
                # dedicated y tile per chunk: the copy never waits on a
                # previous out-DMA (no write-after-read coupling)
                yt = ypool.tile([COUT, CHUNK], BF16, tag=f"y{c}", name=f"ych{c}")
                # one big PSUM->SBUF downconvert copy; alternate engines so
                # consecutive chunks' copies overlap
                if c % 2 == 0:
                    nc.vector.tensor_copy(yt[:], ps[:])
                else:
                    nc.scalar.copy(yt[:], ps[:])
                nc.sync.dma_start(y[:, base: base + CHUNK], yt[:])
    nc.compile()
    return nc


_NC_CACHE = {}


def _get_nc():
    if "nc" not in _NC_CACHE:
        _NC_CACHE["nc"] = _build_nc()
    return _NC_CACHE["nc"]


def make_in_maps(x, alphas):
    """Per-core input maps: bf16, with the (h,w) flip pre-applied to x."""
    x16 = np.asarray(x, dtype=np.float32).astype(NP_BF16)
    wT = np.ascontiguousarray(
        np.asarray(alphas, dtype=np.float32).T
    ).astype(NP_BF16)
    maps = []
    for c in range(N_CORES):
        xf = x16[c][:, _FLIP][:, :, _FLIP]
        maps.append(
            {"x": np.ascontiguousarray(xf.reshape(CIN, HW)), "wT": wT}
        )
    return maps


def kernel(x, alphas, betas=None, **_unused):
    nc = _get_nc()
    in_maps = make_in_maps(x, alphas)
    res = run_bass_kernel_spmd(nc, in_maps, core_ids=list(range(N_CORES)))
    out = np.stack(
        [
            res.results[c]["y"].astype(np.float32).reshape(COUT, H, W)
            for c in range(N_CORES)
        ]
    )
    return out
